# revision 29
# baseline (speedup 1.0000x reference)
"""DeepSeekV3-style MoE layer (1 MoE block) on 8 Trainium2 NeuronCores.

Sharding: expert-parallel. Each core owns 4 of the 32 routed experts and a
64-wide shard of the shared expert's intermediate dim. The router is
replicated (router weight columns are permuted per-core so the local experts
always sit in columns 0..3 — top-k and sigmoid are permutation invariant).
Partial outputs are combined with three on-device ReduceScatters over row
ranges of the output; the first two overlap trailing chunk compute, so only
the last (512-row) one is a tail. The host reassembles the output shards.

v2 changes vs the first working version:
  - all weights and x are pre-cast/pre-laid-out to bf16 on the HOST
    (x shipped as split-bf16 pair x1/x2; Wr as wr1/wr2), so the device does
    no fp32->bf16 casting, no DRAM bounce of x, and DMA-transposes read
    straight from the input tensors
  - shared-expert gate and up projections packed into one PSUM group
    ([128, TC]: partitions 0..63 gate, 64..127 up) halving its matmul count
  - output combined with 3 ReduceScatters (rows 0:2560 after chunk 4,
    2560:3584 after chunk 6, 3584:4096 after chunk 7); x DMA-transposes for
    all later chunks are issued before the first RS so Tile's
    transpose/collective serialization never stalls the PE
"""

import sys

sys.path.insert(0, "/opt/trn_rl_repo")

import numpy as np

import concourse.bacc as bacc
import concourse.bass as bass
import concourse.mybir as mybir
import concourse.tile as tile
from concourse.masks import make_identity

F32 = mybir.dt.float32
BF16 = mybir.dt.bfloat16
AF = mybir.ActivationFunctionType
ALU = mybir.AluOpType

H, I, E, TOPK = 1024, 512, 32, 8
B, S = 4, 1024
T = B * S
NCORES = 8
E_LOC = E // NCORES          # 4 routed experts per core
I_SH = I // NCORES           # 64-wide shared-expert shard per core
ISH2 = 2 * I_SH              # gate+up packed partition count
P = 128
TC = 512                     # token chunk
NCH = T // TC                # 8 chunks
NH = H // P                  # 8 hidden k-tiles
NI = I // P                  # 4 intermediate tiles
NJ = TC // P                 # 4 token tiles per chunk
T_SHARD = T // NCORES        # 512 rows per core after the ReduceScatters
NEG = -1.0e30

# (full-tensor row range, per-core output row range) for the three RSs;
# each fires once its last writer chunk is stored, overlapping later compute
RS_SPLITS = [(0, 2560, 0, 320), (2560, 3584, 320, 448), (3584, 4096, 448, 512)]
RS_AFTER = {4: 0, 6: 1, 7: 2}


def build_nc():
    nc = bacc.Bacc(None, target_bir_lowering=False, num_devices=NCORES)

    x1_d = nc.declare_dram_parameter("x1", [T, H], BF16, isOutput=False)
    x2_d = nc.declare_dram_parameter("x2", [T, H], BF16, isOutput=False)
    # router weights packed [w1 | w2] so one matmul pass computes both terms
    wr12_d = nc.declare_dram_parameter("wr12", [P, NH, 2 * E], BF16,
                                       isOutput=False)
    # [identity64 ; (br | 0)] — moving operand of the logits transpose matmul
    m65_d = nc.declare_dram_parameter("m65", [2 * E + 1, 2 * E], F32,
                                      isOutput=False)
    wg_d = nc.declare_dram_parameter("wg", [E_LOC, P, NH, I], BF16, isOutput=False)
    wu_d = nc.declare_dram_parameter("wu", [E_LOC, P, NH, I], BF16, isOutput=False)
    wd_d = nc.declare_dram_parameter("wd", [E_LOC, P, NI, H], BF16, isOutput=False)
    bg_d = nc.declare_dram_parameter("bg", [P, E_LOC, NI], F32, isOutput=False)
    bu_d = nc.declare_dram_parameter("bu", [P, E_LOC, NI], F32, isOutput=False)
    bias5_d = nc.declare_dram_parameter("bias5", [E_LOC + 1, H], BF16, isOutput=False)
    wgus_d = nc.declare_dram_parameter("wgus", [P, NH, ISH2], BF16, isOutput=False)
    bgus_d = nc.declare_dram_parameter("bgus", [ISH2], F32, isOutput=False)
    wds_d = nc.declare_dram_parameter("wds", [I_SH, H], BF16, isOutput=False)
    sel_d = nc.declare_dram_parameter("sel", [E_LOC, E_LOC * P], BF16, isOutput=False)
    y_d = nc.declare_dram_parameter("y", [T_SHARD, H], F32, isOutput=True)

    # One input tensor per ReduceScatter so writes of later chunks never
    # alias the tensor a running collective is reading (Tile tracks comm
    # input writers at tensor granularity).
    cc_ins = [nc.dram_tensor(f"cc_in{i}", [r1 - r0, H], F32)
              for i, (r0, r1, _, _) in enumerate(RS_SPLITS)]
    cc_out = nc.dram_tensor("cc_out", [T_SHARD, H], F32)

    def cc_slot(row):
        """(tensor, local row) for a global output row."""
        for i, (r0, r1, _, _) in enumerate(RS_SPLITS):
            if r0 <= row < r1:
                return cc_ins[i], row - r0
        raise AssertionError(row)

    with tile.TileContext(nc) as tc:
        with (
            tc.tile_pool(name="wres", bufs=1) as wres,
            tc.tile_pool(name="xtb", bufs=3) as xtb,
            tc.tile_pool(name="xtb2", bufs=2) as xtb2,
            tc.tile_pool(name="hgep", bufs=1) as hgep,
            tc.tile_pool(name="actp", bufs=2) as actp,
            tc.tile_pool(name="outp", bufs=2) as outp,
            tc.tile_pool(name="rtp", bufs=2) as rtp,
            tc.tile_pool(name="ps_tr", bufs=1, space="PSUM") as ps_tr,
            tc.tile_pool(name="ps_r", bufs=1, space="PSUM") as ps_r,
            tc.tile_pool(name="ps_g", bufs=2, space="PSUM") as ps_g,
            tc.tile_pool(name="ps_u", bufs=2, space="PSUM") as ps_u,
            tc.tile_pool(name="ps_d", bufs=1, space="PSUM") as ps_d,
        ):
            # ---------- constants ----------
            ident = wres.tile([P, P], F32, tag="ident")
            make_identity(nc, ident[:])

            def stage_x(ch):
                """DMA-transpose both split-bf16 x streams for one chunk."""
                t0 = ch * TC
                out = {}
                for h in range(NH):
                    xt = xtb.tile([P, TC], BF16, tag=f"xtb{h}", name=f"xtb{h}")
                    nc.sync.dma_start_transpose(
                        xt[:], x1_d[t0:t0 + TC, h * P:(h + 1) * P])
                    xt2 = xtb2.tile([P, TC], BF16, tag=f"xt2{h}", name=f"xt2{h}")
                    nc.sync.dma_start_transpose(
                        xt2[:], x2_d[t0:t0 + TC, h * P:(h + 1) * P])
                    out[h] = (xt, xt2)
                return out

            # chunk 0 x pipeline first so PE work is unblocked early
            tiles = {0: stage_x(0)}

            # ---------- small weights (gpsimd/SWDGE queue: keeps the Sync ring
            # free for x transposes and the Scalar FIFO free for activations) --
            wr12_sb = wres.tile([P, NH, 2 * E], BF16, tag="wr12")
            nc.gpsimd.dma_start(wr12_sb[:], wr12_d[:])
            m65_sb = wres.tile([2 * E + 1, 2 * E], F32, tag="m65")
            nc.gpsimd.dma_start(m65_sb[:], m65_d[:])
            # stationary for the logits transpose: rows 0..63 logits
            # (feature-major, rewritten per chunk), row 64 stays all-ones
            l65_sb = wres.tile([2 * E + 1, TC], F32, tag="l65")
            nc.vector.memset(l65_sb[:], 1.0)
            sel_sb = wres.tile([E_LOC, E_LOC * P], BF16, tag="sel")
            nc.gpsimd.dma_start(sel_sb[:], sel_d[:])
            bg_sb = wres.tile([P, E_LOC, NI], F32, tag="bg")
            nc.gpsimd.dma_start(bg_sb[:], bg_d[:])
            bu_sb = wres.tile([P, E_LOC, NI], F32, tag="bu")
            nc.gpsimd.dma_start(bu_sb[:], bu_d[:])
            bgs_sb = wres.tile([I_SH, 1], F32, tag="bgs")
            nc.gpsimd.dma_start(bgs_sb[:],
                                bgus_d.rearrange("(e o) -> e o", o=1)[0:I_SH])
            bus_sb = wres.tile([I_SH, 1], F32, tag="bus")
            nc.gpsimd.dma_start(bus_sb[:],
                                bgus_d.rearrange("(e o) -> e o", o=1)[I_SH:ISH2])
            bias5_sb = wres.tile([E_LOC + 1, H], BF16, tag="bias5")
            nc.gpsimd.dma_start(bias5_sb[:], bias5_d[:])

            # routing weights, feature-major: rows 0..3 local expert w, row 4 ones
            we_sb = wres.tile([E_LOC + 1, T], BF16, tag="we")
            nc.vector.memset(we_sb[:], 1.0)

            def router(ch, xt):
                t0 = ch * TC
                # one packed pass over x1 and one over x2: rows 0..31 get
                # w1(x1+x2), rows 32..63 get w2(x1+x2) — the extra w2*x2 term
                # is O(1e-5) relative and harmless
                pr = ps_r.tile([2 * E, TC], F32, tag="r", name="pr")
                for h in range(NH):
                    nc.tensor.matmul(pr[:], wr12_sb[:, h, :], xt[h][0][:],
                                     start=(h == 0), stop=False)
                    nc.tensor.matmul(pr[:], wr12_sb[:, h, :], xt[h][1][:],
                                     start=False, stop=(h == NH - 1))
                nc.vector.tensor_copy(l65_sb[0:2 * E, :], pr[:])
                # transpose to token-major and add bias via the ones row:
                # pt[t, e'] = logits_pair[e', t] + (br|0)[e']
                logits_tm = rtp.tile([P, NJ, E], F32, tag="logits_tm")
                for j in range(NJ):
                    pt = ps_tr.tile([P, 2 * E], F32, tag="tr", name="ptl")
                    nc.tensor.matmul(pt[:], l65_sb[:, j * P:(j + 1) * P],
                                     m65_sb[:], start=True, stop=True)
                    lt = rtp.tile([P, 2 * E], F32, tag="lt")
                    nc.vector.tensor_copy(lt[:], pt[:])
                    nc.vector.tensor_tensor(logits_tm[:, j, :], lt[:, 0:E],
                                            lt[:, E:2 * E], ALU.add)
                # top-8 threshold by iterative max extraction
                cur = rtp.tile([P, NJ, E], F32, tag="cur")
                nc.vector.tensor_copy(cur[:], logits_tm[:])
                mx = rtp.tile([P, NJ], F32, tag="mx")
                mask = rtp.tile([P, NJ, E], F32, tag="mask", bufs=1)
                for k in range(TOPK):
                    nc.vector.tensor_reduce(mx[:], cur[:], mybir.AxisListType.X,
                                            ALU.max)
                    if k < TOPK - 1:
                        mxb = mx[:].rearrange("p (f o) -> p f o", o=1).broadcast_to(
                            [P, NJ, E])
                        nc.vector.tensor_tensor(mask[:], cur[:], mxb, ALU.is_ge)
                        nc.vector.scalar_tensor_tensor(cur[:], mask[:], NEG, cur[:],
                                                       ALU.mult, ALU.add)
                # mask8 / normalized sigmoid weights
                aff = rtp.tile([P, NJ, E], F32, tag="aff")
                nc.scalar.activation(aff[:], logits_tm[:], AF.Sigmoid)
                thrb = mx[:].rearrange("p (f o) -> p f o", o=1).broadcast_to(
                    [P, NJ, E])
                nc.vector.tensor_tensor(mask[:], logits_tm[:], thrb, ALU.is_ge)
                nc.vector.tensor_tensor(aff[:], aff[:], mask[:], ALU.mult)
                den = rtp.tile([P, NJ], F32, tag="den")
                nc.vector.tensor_reduce(den[:], aff[:], mybir.AxisListType.X, ALU.add)
                rec = rtp.tile([P, NJ], F32, tag="rec")
                nc.vector.reciprocal(rec[:], den[:])
                recb = rec[:].rearrange("p (f o) -> p f o", o=1).broadcast_to(
                    [P, NJ, E])
                w_tm = rtp.tile([P, NJ, E], F32, tag="w_tm")
                nc.vector.tensor_tensor(w_tm[:], aff[:], recb, ALU.mult)
                # local expert weights, feature-major -> we_sb rows 0..3 (bf16)
                for j in range(NJ):
                    pt = ps_tr.tile([E_LOC, P], F32, tag="tr", name="ptw")
                    nc.tensor.transpose(pt[:], w_tm[:, j, 0:E_LOC], ident[:])
                    nc.vector.tensor_copy(
                        we_sb[0:E_LOC, t0 + j * P:t0 + (j + 1) * P], pt[:])

            router(0, tiles[0])

            # ---------- resident expert weights (pre-cast bf16, direct DMA) ----
            wg_bf = {}
            wu_bf = {}
            wd_bf = {}
            # gate weights on the gpsimd (SWDGE) ring, up weights on the
            # scalar (HWDGE) ring — the two fills run in parallel at startup
            # (plain DMAs only on scalar; transposes stay on sync)
            for e in range(E_LOC):
                for name, dram, store, eng in (
                        ("wg", wg_d, wg_bf, nc.gpsimd),
                        ("wu", wu_d, wu_bf, nc.scalar)):
                    res = wres.tile([P, NH, I], BF16, tag=f"{name}{e}",
                                    name="wres_gu")
                    eng.dma_start(res[:], dram[e])
                    store[e] = res
            wgus_sb = wres.tile([P, NH, ISH2], BF16, tag="wgus")
            nc.gpsimd.dma_start(wgus_sb[:], wgus_d[:])
            for e in range(E_LOC):
                res = wres.tile([P, NI, H], BF16, tag=f"wd{e}", name="wres_d")
                nc.gpsimd.dma_start(res[:], wd_d[e])
                wd_bf[e] = res
            wds_sb = wres.tile([I_SH, H], BF16, tag="wds")
            nc.gpsimd.dma_start(wds_sb[:], wds_d[:])

            def experts(ch, xt, mid=None):
                t0 = ch * TC
                # gate/up -> hge (bf16)
                hge = {}
                for e in range(E_LOC):
                    # broadcast token-weight row -> [128, TC] via selector matmul
                    pw = ps_r.tile([P, TC], F32, tag="r", name="pw")
                    nc.tensor.matmul(pw[:], sel_sb[:, e * P:(e + 1) * P],
                                     we_sb[0:E_LOC, t0:t0 + TC],
                                     start=True, stop=True)
                    w_bc = actp.tile([P, TC], BF16, tag="w_bc", bufs=1)
                    nc.vector.tensor_copy(w_bc[:], pw[:])
                    for i in range(NI):
                        pg = ps_g.tile([P, TC], F32, tag="g")
                        pu = ps_u.tile([P, TC], F32, tag="u")
                        for h in range(NH):
                            nc.tensor.matmul(pg[:],
                                             wg_bf[e][:, h, i * P:(i + 1) * P],
                                             xt[h][0][:], start=(h == 0),
                                             stop=(h == NH - 1))
                        for h in range(NH):
                            nc.tensor.matmul(pu[:],
                                             wu_bf[e][:, h, i * P:(i + 1) * P],
                                             xt[h][0][:], start=(h == 0),
                                             stop=(h == NH - 1))
                        g_act = actp.tile([P, TC], F32, tag="g_act")
                        nc.scalar.activation(g_act[:], pg[:], AF.Silu,
                                             bias=bg_sb[:, e, i:i + 1])
                        u_w = actp.tile([P, TC], F32, tag="u_w")
                        nc.vector.scalar_tensor_tensor(
                            u_w[:], pu[:], bu_sb[:, e, i:i + 1], w_bc[:],
                            ALU.add, ALU.mult)
                        ht = hgep.tile([P, TC], BF16, tag=f"hge{e}_{i}", name="ht")
                        nc.vector.tensor_tensor(ht[:], g_act[:], u_w[:], ALU.mult)
                        hge[(e, i)] = ht

                # shared expert shard -> hge_s (bf16, 64 partitions)
                psg = ps_g.tile([I_SH, TC], F32, tag="g", name="psg")
                psu = ps_u.tile([I_SH, TC], F32, tag="u", name="psu")
                for h in range(NH):
                    nc.tensor.matmul(psg[:], wgus_sb[:, h, 0:I_SH], xt[h][0][:],
                                     start=(h == 0), stop=(h == NH - 1))
                for h in range(NH):
                    nc.tensor.matmul(psu[:], wgus_sb[:, h, I_SH:ISH2], xt[h][0][:],
                                     start=(h == 0), stop=(h == NH - 1))
                gs = actp.tile([I_SH, TC], F32, tag="gs", bufs=1)
                nc.scalar.activation(gs[:], psg[:], AF.Silu, bias=bgs_sb[:, 0:1])
                hs = hgep.tile([I_SH, TC], BF16, tag="hge_s")
                nc.vector.scalar_tensor_tensor(hs[:], psu[:], bus_sb[:, 0:1],
                                               gs[:], ALU.add, ALU.mult)

                # down projection, token-major output
                for j in range(NJ):
                    ts = t0 + j * P
                    out_sb = outp.tile([P, H], F32, tag="out")
                    for half in range(2):
                        hs0 = half * (H // 2)
                        pd = ps_d.tile([P, H // 2], F32, tag=f"d{half}",
                                       name=f"pd{half}")
                        m = 0
                        for e in range(E_LOC):
                            for i in range(NI):
                                nc.tensor.matmul(
                                    pd[:],
                                    hge[(e, i)][:, j * P:(j + 1) * P],
                                    wd_bf[e][:, i, hs0:hs0 + H // 2],
                                    start=(m == 0), stop=False)
                                m += 1
                        nc.tensor.matmul(pd[:],
                                         hs[:, j * P:(j + 1) * P],
                                         wds_sb[:, hs0:hs0 + H // 2],
                                         start=False, stop=False)
                        nc.tensor.matmul(pd[:],
                                         we_sb[:, ts:ts + P],
                                         bias5_sb[:, hs0:hs0 + H // 2],
                                         start=False, stop=True)
                        nc.vector.tensor_copy(out_sb[:, hs0:hs0 + H // 2], pd[:])
                    cc_t, r = cc_slot(ts)
                    nc.scalar.dma_start(cc_t[r:r + P, :], out_sb[:])
                    if mid and j in mid:
                        mid[j]()

            def reduce_split(i):
                r0, r1, o0, o1 = RS_SPLITS[i]
                nc.gpsimd.collective_compute(
                    "ReduceScatter",
                    ALU.add,
                    ins=[cc_ins[i][:]],
                    outs=[cc_out[o0:o1]],
                    replica_groups=[list(range(NCORES))],
                )
                # y store on gpsimd: a y store waits on its RS, and on the
                # Scalar FIFO that wait would stall the next chunks' SILUs
                nc.gpsimd.dma_start(y_d[o0:o1], cc_out[o0:o1])

            # ---------- main loop ----------
            # Staging runs two chunks ahead (so the PE never waits on a
            # not-yet-transposed x tile), EXCEPT that chunk 7's staging is
            # pulled into iteration 4 so every DMA-transpose is issued before
            # the first ReduceScatter (Tile serializes transposes against
            # collectives; a transpose issued after an RS waits for it).
            tiles[1] = stage_x(1)
            router(1, tiles[1])
            for ch in range(NCH):
                # Staging runs two chunks ahead, except chunk 7's staging is
                # pulled into iteration 4 so every DMA-transpose is issued
                # before the first ReduceScatter (Tile serializes transposes
                # against collectives).
                if ch + 2 < NCH and ch != 5:
                    tiles[ch + 2] = stage_x(ch + 2)
                experts(ch, tiles.pop(ch))
                if ch + 2 < NCH:
                    router(ch + 2, tiles[ch + 2])
                if ch == 4:
                    tiles[7] = stage_x(7)
                if ch in RS_AFTER:
                    reduce_split(RS_AFTER[ch])

    nc.finalize()
    return nc


def prep_inputs(inputs):
    """Split/replicate/bf16-cast full inputs into 8 per-core input maps."""
    import ml_dtypes
    bf = ml_dtypes.bfloat16

    hs = np.ascontiguousarray(np.asarray(inputs["hidden_states"], dtype=np.float32))
    x = hs.reshape(T, H)
    x1 = x.astype(bf)
    x2 = (x - x1.astype(np.float32)).astype(bf)
    Wr = np.asarray(inputs["Wr"], np.float32)
    br = np.asarray(inputs["br"], np.float32)
    Wg = np.asarray(inputs["Wg"], np.float32)
    bg = np.asarray(inputs["bg"], np.float32)
    Wu = np.asarray(inputs["Wu"], np.float32)
    bu = np.asarray(inputs["bu"], np.float32)
    Wd = np.asarray(inputs["Wd"], np.float32)
    bd = np.asarray(inputs["bd"], np.float32)
    Wg_s = np.asarray(inputs["Wg_s"], np.float32)
    bg_s = np.asarray(inputs["bg_s"], np.float32)
    Wu_s = np.asarray(inputs["Wu_s"], np.float32)
    bu_s = np.asarray(inputs["bu_s"], np.float32)
    Wd_s = np.asarray(inputs["Wd_s"], np.float32)
    bd_s = np.asarray(inputs["bd_s"], np.float32)

    sel = np.kron(np.eye(E_LOC, dtype=np.float32),
                  np.ones((1, P), dtype=np.float32)).astype(bf)

    in_maps = []
    for c in range(NCORES):
        loc = list(range(c * E_LOC, (c + 1) * E_LOC))
        rest = [e for e in range(E) if e not in loc]
        perm = loc + rest
        sh = slice(c * I_SH, (c + 1) * I_SH)

        Wr_p = np.ascontiguousarray(Wr[:, perm])
        wr1 = Wr_p.astype(bf)
        wr2 = (Wr_p - wr1.astype(np.float32)).astype(bf)
        wr12 = np.concatenate([wr1, wr2], axis=1)        # [H, 2E] bf16
        m65 = np.zeros((2 * E + 1, 2 * E), np.float32)
        m65[0:2 * E, 0:2 * E] = np.eye(2 * E)
        m65[2 * E, 0:E] = br[perm]

        bias5 = np.concatenate(
            [bd[loc], (bd_s if c == 0 else np.zeros_like(bd_s))[None, :]],
            axis=0).astype(bf)
        wgus = np.concatenate([Wg_s[:, sh], Wu_s[:, sh]], axis=1)  # [H, 128]
        in_maps.append({
            "x1": x1,
            "x2": x2,
            "wr12": np.ascontiguousarray(
                wr12.reshape(NH, P, 2 * E).transpose(1, 0, 2)),
            "m65": m65,
            "wg": np.ascontiguousarray(
                Wg[loc].reshape(E_LOC, NH, P, I).transpose(0, 2, 1, 3).astype(bf)),
            "wu": np.ascontiguousarray(
                Wu[loc].reshape(E_LOC, NH, P, I).transpose(0, 2, 1, 3).astype(bf)),
            "wd": np.ascontiguousarray(
                Wd[loc].reshape(E_LOC, NI, P, H).transpose(0, 2, 1, 3).astype(bf)),
            "bg": np.ascontiguousarray(bg[loc].reshape(E_LOC, NI, P).transpose(2, 0, 1)),
            "bu": np.ascontiguousarray(bu[loc].reshape(E_LOC, NI, P).transpose(2, 0, 1)),
            "bias5": np.ascontiguousarray(bias5),
            "wgus": np.ascontiguousarray(
                wgus.reshape(NH, P, ISH2).transpose(1, 0, 2).astype(bf)),
            "bgus": np.ascontiguousarray(np.concatenate([bg_s[sh], bu_s[sh]])),
            "wds": np.ascontiguousarray(Wd_s[sh, :].astype(bf)),
            "sel": sel,
        })
    return in_maps


def assemble_output(results):
    """Reassemble [T, H] from the three per-core ReduceScatter shards."""
    out = np.empty((T, H), np.float32)
    for c in range(NCORES):
        y = results[c]["y"]
        for r0, r1, o0, o1 in RS_SPLITS:
            n = (r1 - r0) // NCORES
            out[r0 + c * n:r0 + (c + 1) * n] = y[o0:o1]
    return out


_CACHE = {}


def get_runner():
    """Build + jit once; returns run(in_maps) -> list of per-core output dicts."""
    if "run" in _CACHE:
        return _CACHE["run"]
    import jax
    from jax.sharding import Mesh, PartitionSpec
    from jax.experimental.shard_map import shard_map
    from concourse import bass2jax

    nc = build_nc()
    bass2jax.install_neuronx_cc_hook()

    in_names = []
    out_names = []
    out_avals = []
    partition_name = nc.partition_id_tensor.name if nc.partition_id_tensor else None
    for alloc in nc.m.functions[0].allocations:
        if not isinstance(alloc, mybir.MemoryLocationSet):
            continue
        name = alloc.memorylocations[0].name
        if alloc.kind == "ExternalInput":
            if name != partition_name:
                in_names.append(name)
        elif alloc.kind == "ExternalOutput":
            out_names.append(name)
            out_avals.append(
                jax.core.ShapedArray(tuple(alloc.tensor_shape),
                                     mybir.dt.np(alloc.dtype)))
    n_params = len(in_names)
    n_outs = len(out_names)
    all_names = in_names + out_names + ([partition_name] if partition_name else [])
    donate = tuple(range(n_params, n_params + n_outs))

    def _body(*args):
        operands = list(args)
        if partition_name is not None:
            operands.append(bass2jax.partition_id_tensor())
        return tuple(bass2jax._bass_exec_p.bind(
            *operands,
            out_avals=tuple(out_avals),
            in_names=tuple(all_names),
            out_names=tuple(out_names),
            lowering_input_output_aliases=(),
            sim_require_finite=True,
            sim_require_nnan=True,
            nc=nc,
        ))

    devices = jax.devices()[:NCORES]
    mesh = Mesh(np.asarray(devices), ("core",))
    in_specs = (PartitionSpec("core"),) * (n_params + n_outs)
    out_specs = (PartitionSpec("core"),) * n_outs
    sharded = jax.jit(
        shard_map(_body, mesh=mesh, in_specs=in_specs, out_specs=out_specs,
                  check_rep=False),
        donate_argnums=donate, keep_unused=True)

    def run(in_maps, dev_inputs=None):
        if dev_inputs is None:
            dev_inputs = [
                np.concatenate([np.asarray(in_maps[c][n]) for c in range(NCORES)],
                               axis=0)
                for n in in_names
            ]
        zeros = [np.zeros((NCORES * a.shape[0], *a.shape[1:]), a.dtype)
                 for a in out_avals]
        outs = sharded(*dev_inputs, *zeros)
        return [
            {name: np.asarray(outs[i]).reshape(NCORES, *out_avals[i].shape)[c]
             for i, name in enumerate(out_names)}
            for c in range(NCORES)
        ]

    _CACHE["run"] = run
    _CACHE["meta"] = (in_names, out_names, out_avals, sharded, mesh)
    return run


def kernel(**inputs) -> np.ndarray:
    run = get_runner()
    in_maps = prep_inputs(inputs)
    results = run(in_maps)
    return assemble_output(results).reshape(B, S, H).astype(np.float32)


# revision 39
# speedup vs baseline: 1.0116x; 1.0116x over previous
"""DeepSeekV3-style MoE layer (1 MoE block) on 8 Trainium2 NeuronCores.

Sharding: expert-parallel. Each core owns 4 of the 32 routed experts and a
64-wide shard of the shared expert's intermediate dim. The router is
replicated (router weight columns are permuted per-core so the local experts
always sit in columns 0..3 — top-k and sigmoid are permutation invariant).
Partial outputs are combined with three on-device ReduceScatters over row
ranges of the output; the first two overlap trailing chunk compute, so only
the last (512-row) one is a tail. The host reassembles the output shards.

v2 changes vs the first working version:
  - all weights and x are pre-cast/pre-laid-out to bf16 on the HOST
    (x shipped as split-bf16 pair x1/x2; Wr as wr1/wr2), so the device does
    no fp32->bf16 casting, no DRAM bounce of x, and DMA-transposes read
    straight from the input tensors
  - shared-expert gate and up projections packed into one PSUM group
    ([128, TC]: partitions 0..63 gate, 64..127 up) halving its matmul count
  - output combined with 3 ReduceScatters (rows 0:2560 after chunk 4,
    2560:3584 after chunk 6, 3584:4096 after chunk 7); x DMA-transposes for
    all later chunks are issued before the first RS so Tile's
    transpose/collective serialization never stalls the PE
"""

import sys

sys.path.insert(0, "/opt/trn_rl_repo")

import numpy as np

import concourse.bacc as bacc
import concourse.bass as bass
import concourse.mybir as mybir
import concourse.tile as tile
from concourse.masks import make_identity

F32 = mybir.dt.float32
BF16 = mybir.dt.bfloat16
AF = mybir.ActivationFunctionType
ALU = mybir.AluOpType

H, I, E, TOPK = 1024, 512, 32, 8
B, S = 4, 1024
T = B * S
NCORES = 8
E_LOC = E // NCORES          # 4 routed experts per core
I_SH = I // NCORES           # 64-wide shared-expert shard per core
ISH2 = 2 * I_SH              # gate+up packed partition count
P = 128
TC = 512                     # token chunk
NCH = T // TC                # 8 chunks
NH = H // P                  # 8 hidden k-tiles
NI = I // P                  # 4 intermediate tiles
NJ = TC // P                 # 4 token tiles per chunk
T_SHARD = T // NCORES        # 512 rows per core after the ReduceScatters
NEG = -1.0e30

# (full-tensor row range, per-core output row range) for the three RSs;
# each fires once its last writer chunk is stored, overlapping later compute
RS_SPLITS = [(0, 2560, 0, 320), (2560, 3584, 320, 448), (3584, 4096, 448, 512)]
RS_AFTER = {4: 0, 6: 1, 7: 2}


def build_nc():
    nc = bacc.Bacc(None, target_bir_lowering=False, num_devices=NCORES)

    x1_d = nc.declare_dram_parameter("x1", [T, H], BF16, isOutput=False)
    x2_d = nc.declare_dram_parameter("x2", [T, H], BF16, isOutput=False)
    # router weights packed [w1 | w2] so one matmul pass computes both terms
    wr12_d = nc.declare_dram_parameter("wr12", [P, NH, 2 * E], BF16,
                                       isOutput=False)
    # [identity64 ; (br | 0)] — moving operand of the logits transpose matmul
    m65_d = nc.declare_dram_parameter("m65", [2 * E + 1, 2 * E], F32,
                                      isOutput=False)
    wg_d = nc.declare_dram_parameter("wg", [E_LOC, P, NH, I], BF16, isOutput=False)
    wu_d = nc.declare_dram_parameter("wu", [E_LOC, P, NH, I], BF16, isOutput=False)
    wd_d = nc.declare_dram_parameter("wd", [E_LOC, P, NI, H], BF16, isOutput=False)
    bg_d = nc.declare_dram_parameter("bg", [P, E_LOC, NI], F32, isOutput=False)
    bu_d = nc.declare_dram_parameter("bu", [P, E_LOC, NI], F32, isOutput=False)
    wgus_d = nc.declare_dram_parameter("wgus", [P, NH, ISH2], BF16, isOutput=False)
    bgus_d = nc.declare_dram_parameter("bgus", [ISH2], F32, isOutput=False)
    # shared-expert down weights with the 5 bias rows appended: the down
    # matmul's stationary carries [hge_s ; w_e rows ; ones] so one matmul
    # does shared-down + the per-token bias combine
    wdsb5_d = nc.declare_dram_parameter("wdsb5", [I_SH + E_LOC + 1, H], BF16,
                                        isOutput=False)
    sel_d = nc.declare_dram_parameter("sel", [E_LOC, E_LOC * P], BF16, isOutput=False)
    y_d = nc.declare_dram_parameter("y", [T_SHARD, H], F32, isOutput=True)

    # One input tensor per ReduceScatter so writes of later chunks never
    # alias the tensor a running collective is reading (Tile tracks comm
    # input writers at tensor granularity).
    cc_ins = [nc.dram_tensor(f"cc_in{i}", [r1 - r0, H], F32)
              for i, (r0, r1, _, _) in enumerate(RS_SPLITS)]
    cc_out = nc.dram_tensor("cc_out", [T_SHARD, H], F32)
    # routing-weight rows bounced through DRAM so they can be DMAed into
    # partitions 64..68 of the shared-expert stationary (DVE can't write
    # cross-partition-base; DMA can)
    we_dram = nc.dram_tensor("we_dram", [E_LOC + 1, T], BF16)

    def cc_slot(row):
        """(tensor, local row) for a global output row."""
        for i, (r0, r1, _, _) in enumerate(RS_SPLITS):
            if r0 <= row < r1:
                return cc_ins[i], row - r0
        raise AssertionError(row)

    with tile.TileContext(nc) as tc:
        with (
            tc.tile_pool(name="wres", bufs=1) as wres,
            tc.tile_pool(name="xtb", bufs=3) as xtb,
            tc.tile_pool(name="xtb2", bufs=2) as xtb2,
            tc.tile_pool(name="hgep", bufs=1) as hgep,
            tc.tile_pool(name="actp", bufs=2) as actp,
            tc.tile_pool(name="outp", bufs=2) as outp,
            tc.tile_pool(name="rtp", bufs=2) as rtp,
            tc.tile_pool(name="ps_tr", bufs=1, space="PSUM") as ps_tr,
            tc.tile_pool(name="ps_r", bufs=1, space="PSUM") as ps_r,
            tc.tile_pool(name="ps_g", bufs=2, space="PSUM") as ps_g,
            tc.tile_pool(name="ps_u", bufs=2, space="PSUM") as ps_u,
            tc.tile_pool(name="ps_d", bufs=1, space="PSUM") as ps_d,
        ):
            # ---------- constants ----------
            ident = wres.tile([P, P], F32, tag="ident")
            make_identity(nc, ident[:])

            def stage_x(ch):
                """DMA-transpose both split-bf16 x streams for one chunk."""
                t0 = ch * TC
                out = {}
                for h in range(NH):
                    xt = xtb.tile([P, TC], BF16, tag=f"xtb{h}", name=f"xtb{h}")
                    nc.sync.dma_start_transpose(
                        xt[:], x1_d[t0:t0 + TC, h * P:(h + 1) * P])
                    xt2 = xtb2.tile([P, TC], BF16, tag=f"xt2{h}", name=f"xt2{h}")
                    nc.sync.dma_start_transpose(
                        xt2[:], x2_d[t0:t0 + TC, h * P:(h + 1) * P])
                    out[h] = (xt, xt2)
                return out

            # chunk 0 x pipeline first so PE work is unblocked early
            tiles = {0: stage_x(0)}

            # ---------- small weights (gpsimd/SWDGE queue: keeps the Sync ring
            # free for x transposes and the Scalar FIFO free for activations) --
            wr12_sb = wres.tile([P, NH, 2 * E], BF16, tag="wr12")
            nc.gpsimd.dma_start(wr12_sb[:], wr12_d[:])
            m65_sb = wres.tile([2 * E + 1, 2 * E], F32, tag="m65")
            nc.gpsimd.dma_start(m65_sb[:], m65_d[:])
            # stationary for the logits transpose: rows 0..63 logits
            # (feature-major, rewritten per chunk), row 64 stays all-ones
            l65_sb = wres.tile([2 * E + 1, TC], F32, tag="l65")
            nc.vector.memset(l65_sb[:], 1.0)
            sel_sb = wres.tile([E_LOC, E_LOC * P], BF16, tag="sel")
            nc.gpsimd.dma_start(sel_sb[:], sel_d[:])
            bg_sb = wres.tile([P, E_LOC, NI], F32, tag="bg")
            nc.gpsimd.dma_start(bg_sb[:], bg_d[:])
            bu_sb = wres.tile([P, E_LOC, NI], F32, tag="bu")
            nc.gpsimd.dma_start(bu_sb[:], bu_d[:])
            bgs_sb = wres.tile([I_SH, 1], F32, tag="bgs")
            nc.gpsimd.dma_start(bgs_sb[:],
                                bgus_d.rearrange("(e o) -> e o", o=1)[0:I_SH])
            bus_sb = wres.tile([I_SH, 1], F32, tag="bus")
            nc.gpsimd.dma_start(bus_sb[:],
                                bgus_d.rearrange("(e o) -> e o", o=1)[I_SH:ISH2])
            wdsb5_sb = wres.tile([I_SH + E_LOC + 1, H], BF16, tag="wdsb5")
            nc.gpsimd.dma_start(wdsb5_sb[:], wdsb5_d[:])

            # routing weights, feature-major: rows 0..3 local expert w, row 4 ones
            we_sb = wres.tile([E_LOC + 1, T], BF16, tag="we")
            nc.vector.memset(we_sb[:], 1.0)

            def router(ch, xt):
                t0 = ch * TC
                # one packed pass over x1 and one over x2: rows 0..31 get
                # w1(x1+x2), rows 32..63 get w2(x1+x2) — the extra w2*x2 term
                # is O(1e-5) relative and harmless
                pr = ps_r.tile([2 * E, TC], F32, tag="r", name="pr")
                for h in range(NH):
                    nc.tensor.matmul(pr[:], wr12_sb[:, h, :], xt[h][0][:],
                                     start=(h == 0), stop=False)
                    nc.tensor.matmul(pr[:], wr12_sb[:, h, :], xt[h][1][:],
                                     start=False, stop=(h == NH - 1))
                nc.vector.tensor_copy(l65_sb[0:2 * E, :], pr[:])
                # transpose to token-major and add bias via the ones row:
                # pt[t, e'] = logits_pair[e', t] + (br|0)[e']
                logits_tm = rtp.tile([P, NJ, E], F32, tag="logits_tm")
                for j in range(NJ):
                    pt = ps_tr.tile([P, 2 * E], F32, tag="tr", name="ptl")
                    nc.tensor.matmul(pt[:], l65_sb[:, j * P:(j + 1) * P],
                                     m65_sb[:], start=True, stop=True)
                    lt = rtp.tile([P, 2 * E], F32, tag="lt")
                    nc.vector.tensor_copy(lt[:], pt[:])
                    nc.vector.tensor_tensor(logits_tm[:, j, :], lt[:, 0:E],
                                            lt[:, E:2 * E], ALU.add)
                # top-8 threshold by iterative max extraction
                cur = rtp.tile([P, NJ, E], F32, tag="cur")
                nc.vector.tensor_copy(cur[:], logits_tm[:])
                mx = rtp.tile([P, NJ], F32, tag="mx")
                mask = rtp.tile([P, NJ, E], F32, tag="mask", bufs=1)
                for k in range(TOPK):
                    nc.vector.tensor_reduce(mx[:], cur[:], mybir.AxisListType.X,
                                            ALU.max)
                    if k < TOPK - 1:
                        mxb = mx[:].rearrange("p (f o) -> p f o", o=1).broadcast_to(
                            [P, NJ, E])
                        nc.vector.tensor_tensor(mask[:], cur[:], mxb, ALU.is_ge)
                        nc.vector.scalar_tensor_tensor(cur[:], mask[:], NEG, cur[:],
                                                       ALU.mult, ALU.add)
                # mask8 / normalized sigmoid weights
                aff = rtp.tile([P, NJ, E], F32, tag="aff")
                nc.scalar.activation(aff[:], logits_tm[:], AF.Sigmoid)
                thrb = mx[:].rearrange("p (f o) -> p f o", o=1).broadcast_to(
                    [P, NJ, E])
                nc.vector.tensor_tensor(mask[:], logits_tm[:], thrb, ALU.is_ge)
                nc.vector.tensor_tensor(aff[:], aff[:], mask[:], ALU.mult)
                den = rtp.tile([P, NJ], F32, tag="den")
                nc.vector.tensor_reduce(den[:], aff[:], mybir.AxisListType.X, ALU.add)
                rec = rtp.tile([P, NJ], F32, tag="rec")
                nc.vector.reciprocal(rec[:], den[:])
                recb = rec[:].rearrange("p (f o) -> p f o", o=1).broadcast_to(
                    [P, NJ, E])
                w_tm = rtp.tile([P, NJ, E], F32, tag="w_tm")
                nc.vector.tensor_tensor(w_tm[:], aff[:], recb, ALU.mult)
                # local expert weights, feature-major -> we_sb rows 0..3 (bf16)
                for j in range(NJ):
                    pt = ps_tr.tile([E_LOC, P], F32, tag="tr", name="ptw")
                    nc.tensor.transpose(pt[:], w_tm[:, j, 0:E_LOC], ident[:])
                    nc.vector.tensor_copy(
                        we_sb[0:E_LOC, t0 + j * P:t0 + (j + 1) * P], pt[:])
                # bounce this chunk's routing rows (+ones row) to DRAM so
                # experts() can DMA them into the shared stationary's
                # partitions 64..68
                nc.gpsimd.dma_start(we_dram[:, t0:t0 + TC],
                                    we_sb[:, t0:t0 + TC])

            router(0, tiles[0])

            # ---------- resident expert weights (pre-cast bf16, direct DMA) ----
            wg_bf = {}
            wu_bf = {}
            wd_bf = {}
            # gate weights on the gpsimd (SWDGE) ring, up weights on the
            # scalar (HWDGE) ring — the two fills run in parallel at startup
            # (plain DMAs only on scalar; transposes stay on sync)
            for e in range(E_LOC):
                for name, dram, store, eng in (
                        ("wg", wg_d, wg_bf, nc.gpsimd),
                        ("wu", wu_d, wu_bf, nc.scalar)):
                    res = wres.tile([P, NH, I], BF16, tag=f"{name}{e}",
                                    name="wres_gu")
                    eng.dma_start(res[:], dram[e])
                    store[e] = res
            wgus_sb = wres.tile([P, NH, ISH2], BF16, tag="wgus")
            nc.gpsimd.dma_start(wgus_sb[:], wgus_d[:])
            for e in range(E_LOC):
                res = wres.tile([P, NI, H], BF16, tag=f"wd{e}", name="wres_d")
                nc.scalar.dma_start(res[:], wd_d[e])
                wd_bf[e] = res

            def experts(ch, xt, mid=None):
                t0 = ch * TC
                # gate/up -> hge (bf16)
                hge = {}
                for e in range(E_LOC):
                    # broadcast token-weight row -> [128, TC] via selector matmul
                    pw = ps_r.tile([P, TC], F32, tag="r", name="pw")
                    nc.tensor.matmul(pw[:], sel_sb[:, e * P:(e + 1) * P],
                                     we_sb[0:E_LOC, t0:t0 + TC],
                                     start=True, stop=True)
                    w_bc = actp.tile([P, TC], BF16, tag="w_bc", bufs=1)
                    nc.vector.tensor_copy(w_bc[:], pw[:])
                    for i in range(NI):
                        pg = ps_g.tile([P, TC], F32, tag="g")
                        pu = ps_u.tile([P, TC], F32, tag="u")
                        for h in range(NH):
                            nc.tensor.matmul(pg[:],
                                             wg_bf[e][:, h, i * P:(i + 1) * P],
                                             xt[h][0][:], start=(h == 0),
                                             stop=(h == NH - 1))
                        for h in range(NH):
                            nc.tensor.matmul(pu[:],
                                             wu_bf[e][:, h, i * P:(i + 1) * P],
                                             xt[h][0][:], start=(h == 0),
                                             stop=(h == NH - 1))
                        g_act = actp.tile([P, TC], F32, tag="g_act")
                        nc.scalar.activation(g_act[:], pg[:], AF.Silu,
                                             bias=bg_sb[:, e, i:i + 1])
                        u_w = actp.tile([P, TC], F32, tag="u_w")
                        nc.vector.scalar_tensor_tensor(
                            u_w[:], pu[:], bu_sb[:, e, i:i + 1], w_bc[:],
                            ALU.add, ALU.mult)
                        ht = hgep.tile([P, TC], BF16, tag=f"hge{e}_{i}", name="ht")
                        nc.vector.tensor_tensor(ht[:], g_act[:], u_w[:], ALU.mult)
                        hge[(e, i)] = ht

                # shared expert shard -> hge_s (bf16, 64 partitions)
                psg = ps_g.tile([I_SH, TC], F32, tag="g", name="psg")
                psu = ps_u.tile([I_SH, TC], F32, tag="u", name="psu")
                for h in range(NH):
                    nc.tensor.matmul(psg[:], wgus_sb[:, h, 0:I_SH], xt[h][0][:],
                                     start=(h == 0), stop=(h == NH - 1))
                for h in range(NH):
                    nc.tensor.matmul(psu[:], wgus_sb[:, h, I_SH:ISH2], xt[h][0][:],
                                     start=(h == 0), stop=(h == NH - 1))
                gs = actp.tile([I_SH, TC], F32, tag="gs", bufs=1)
                nc.scalar.activation(gs[:], psg[:], AF.Silu, bias=bgs_sb[:, 0:1])
                # shared stationary [69, TC]: rows 0..63 shared hge (DVE),
                # rows 64..68 routing rows via DMA (partition-offset target)
                hs = hgep.tile([I_SH + E_LOC + 1, TC], BF16, tag="hge_s")
                nc.gpsimd.dma_start(hs[I_SH:I_SH + E_LOC + 1, :],
                                    we_dram[:, t0:t0 + TC])
                nc.vector.scalar_tensor_tensor(hs[0:I_SH, :], psu[:],
                                               bus_sb[:, 0:1],
                                               gs[:], ALU.add, ALU.mult)

                # down projection, token-major output
                for j in range(NJ):
                    ts = t0 + j * P
                    out_sb = outp.tile([P, H], F32, tag="out")
                    for half in range(2):
                        hs0 = half * (H // 2)
                        pd = ps_d.tile([P, H // 2], F32, tag=f"d{half}",
                                       name=f"pd{half}")
                        m = 0
                        for e in range(E_LOC):
                            for i in range(NI):
                                nc.tensor.matmul(
                                    pd[:],
                                    hge[(e, i)][:, j * P:(j + 1) * P],
                                    wd_bf[e][:, i, hs0:hs0 + H // 2],
                                    start=(m == 0), stop=False)
                                m += 1
                        nc.tensor.matmul(pd[:],
                                         hs[:, j * P:(j + 1) * P],
                                         wdsb5_sb[:, hs0:hs0 + H // 2],
                                         start=False, stop=True)
                        nc.vector.tensor_copy(out_sb[:, hs0:hs0 + H // 2], pd[:])
                    cc_t, r = cc_slot(ts)
                    nc.scalar.dma_start(cc_t[r:r + P, :], out_sb[:])
                    if mid and j in mid:
                        mid[j]()

            def reduce_split(i):
                r0, r1, o0, o1 = RS_SPLITS[i]
                nc.gpsimd.collective_compute(
                    "ReduceScatter",
                    ALU.add,
                    ins=[cc_ins[i][:]],
                    outs=[cc_out[o0:o1]],
                    replica_groups=[list(range(NCORES))],
                )
                # y store on sync: it waits on its RS, and the sync ring is
                # idle once all transposes are staged (by iteration 4), so
                # nothing queues behind the wait. (On scalar it would stall
                # SILUs; on gpsimd it would stall the we/hs routing-row DMAs.)
                nc.sync.dma_start(y_d[o0:o1], cc_out[o0:o1])

            # ---------- main loop ----------
            # Staging runs two chunks ahead (so the PE never waits on a
            # not-yet-transposed x tile), EXCEPT that chunk 7's staging is
            # pulled into iteration 4 so every DMA-transpose is issued before
            # the first ReduceScatter (Tile serializes transposes against
            # collectives; a transpose issued after an RS waits for it).
            tiles[1] = stage_x(1)
            router(1, tiles[1])
            for ch in range(NCH):
                # Staging runs two chunks ahead, except chunk 7's staging is
                # pulled into iteration 4 so every DMA-transpose is issued
                # before the first ReduceScatter (Tile serializes transposes
                # against collectives).
                if ch + 2 < NCH and ch != 5:
                    tiles[ch + 2] = stage_x(ch + 2)
                experts(ch, tiles.pop(ch))
                if ch + 2 < NCH:
                    router(ch + 2, tiles[ch + 2])
                if ch == 4:
                    tiles[7] = stage_x(7)
                if ch in RS_AFTER:
                    reduce_split(RS_AFTER[ch])

    nc.finalize()
    return nc


def prep_inputs(inputs):
    """Split/replicate/bf16-cast full inputs into 8 per-core input maps."""
    import ml_dtypes
    bf = ml_dtypes.bfloat16

    hs = np.ascontiguousarray(np.asarray(inputs["hidden_states"], dtype=np.float32))
    x = hs.reshape(T, H)
    x1 = x.astype(bf)
    x2 = (x - x1.astype(np.float32)).astype(bf)
    Wr = np.asarray(inputs["Wr"], np.float32)
    br = np.asarray(inputs["br"], np.float32)
    Wg = np.asarray(inputs["Wg"], np.float32)
    bg = np.asarray(inputs["bg"], np.float32)
    Wu = np.asarray(inputs["Wu"], np.float32)
    bu = np.asarray(inputs["bu"], np.float32)
    Wd = np.asarray(inputs["Wd"], np.float32)
    bd = np.asarray(inputs["bd"], np.float32)
    Wg_s = np.asarray(inputs["Wg_s"], np.float32)
    bg_s = np.asarray(inputs["bg_s"], np.float32)
    Wu_s = np.asarray(inputs["Wu_s"], np.float32)
    bu_s = np.asarray(inputs["bu_s"], np.float32)
    Wd_s = np.asarray(inputs["Wd_s"], np.float32)
    bd_s = np.asarray(inputs["bd_s"], np.float32)

    sel = np.kron(np.eye(E_LOC, dtype=np.float32),
                  np.ones((1, P), dtype=np.float32)).astype(bf)

    in_maps = []
    for c in range(NCORES):
        loc = list(range(c * E_LOC, (c + 1) * E_LOC))
        rest = [e for e in range(E) if e not in loc]
        perm = loc + rest
        sh = slice(c * I_SH, (c + 1) * I_SH)

        Wr_p = np.ascontiguousarray(Wr[:, perm])
        wr1 = Wr_p.astype(bf)
        wr2 = (Wr_p - wr1.astype(np.float32)).astype(bf)
        wr12 = np.concatenate([wr1, wr2], axis=1)        # [H, 2E] bf16
        m65 = np.zeros((2 * E + 1, 2 * E), np.float32)
        m65[0:2 * E, 0:2 * E] = np.eye(2 * E)
        m65[2 * E, 0:E] = br[perm]

        bias5 = np.concatenate(
            [bd[loc], (bd_s if c == 0 else np.zeros_like(bd_s))[None, :]],
            axis=0).astype(bf)
        wgus = np.concatenate([Wg_s[:, sh], Wu_s[:, sh]], axis=1)  # [H, 128]
        in_maps.append({
            "x1": x1,
            "x2": x2,
            "wr12": np.ascontiguousarray(
                wr12.reshape(NH, P, 2 * E).transpose(1, 0, 2)),
            "m65": m65,
            "wg": np.ascontiguousarray(
                Wg[loc].reshape(E_LOC, NH, P, I).transpose(0, 2, 1, 3).astype(bf)),
            "wu": np.ascontiguousarray(
                Wu[loc].reshape(E_LOC, NH, P, I).transpose(0, 2, 1, 3).astype(bf)),
            "wd": np.ascontiguousarray(
                Wd[loc].reshape(E_LOC, NI, P, H).transpose(0, 2, 1, 3).astype(bf)),
            "bg": np.ascontiguousarray(bg[loc].reshape(E_LOC, NI, P).transpose(2, 0, 1)),
            "bu": np.ascontiguousarray(bu[loc].reshape(E_LOC, NI, P).transpose(2, 0, 1)),
            "wdsb5": np.ascontiguousarray(
                np.concatenate([Wd_s[sh, :].astype(bf), bias5], axis=0)),
            "wgus": np.ascontiguousarray(
                wgus.reshape(NH, P, ISH2).transpose(1, 0, 2).astype(bf)),
            "bgus": np.ascontiguousarray(np.concatenate([bg_s[sh], bu_s[sh]])),
            "sel": sel,
        })
    return in_maps


def assemble_output(results):
    """Reassemble [T, H] from the three per-core ReduceScatter shards."""
    out = np.empty((T, H), np.float32)
    for c in range(NCORES):
        y = results[c]["y"]
        for r0, r1, o0, o1 in RS_SPLITS:
            n = (r1 - r0) // NCORES
            out[r0 + c * n:r0 + (c + 1) * n] = y[o0:o1]
    return out


_CACHE = {}


def get_runner():
    """Build + jit once; returns run(in_maps) -> list of per-core output dicts."""
    if "run" in _CACHE:
        return _CACHE["run"]
    import jax
    from jax.sharding import Mesh, PartitionSpec
    from jax.experimental.shard_map import shard_map
    from concourse import bass2jax

    nc = build_nc()
    bass2jax.install_neuronx_cc_hook()

    in_names = []
    out_names = []
    out_avals = []
    partition_name = nc.partition_id_tensor.name if nc.partition_id_tensor else None
    for alloc in nc.m.functions[0].allocations:
        if not isinstance(alloc, mybir.MemoryLocationSet):
            continue
        name = alloc.memorylocations[0].name
        if alloc.kind == "ExternalInput":
            if name != partition_name:
                in_names.append(name)
        elif alloc.kind == "ExternalOutput":
            out_names.append(name)
            out_avals.append(
                jax.core.ShapedArray(tuple(alloc.tensor_shape),
                                     mybir.dt.np(alloc.dtype)))
    n_params = len(in_names)
    n_outs = len(out_names)
    all_names = in_names + out_names + ([partition_name] if partition_name else [])
    donate = tuple(range(n_params, n_params + n_outs))

    def _body(*args):
        operands = list(args)
        if partition_name is not None:
            operands.append(bass2jax.partition_id_tensor())
        return tuple(bass2jax._bass_exec_p.bind(
            *operands,
            out_avals=tuple(out_avals),
            in_names=tuple(all_names),
            out_names=tuple(out_names),
            lowering_input_output_aliases=(),
            sim_require_finite=True,
            sim_require_nnan=True,
            nc=nc,
        ))

    devices = jax.devices()[:NCORES]
    mesh = Mesh(np.asarray(devices), ("core",))
    in_specs = (PartitionSpec("core"),) * (n_params + n_outs)
    out_specs = (PartitionSpec("core"),) * n_outs
    sharded = jax.jit(
        shard_map(_body, mesh=mesh, in_specs=in_specs, out_specs=out_specs,
                  check_rep=False),
        donate_argnums=donate, keep_unused=True)

    def run(in_maps, dev_inputs=None):
        if dev_inputs is None:
            dev_inputs = [
                np.concatenate([np.asarray(in_maps[c][n]) for c in range(NCORES)],
                               axis=0)
                for n in in_names
            ]
        zeros = [np.zeros((NCORES * a.shape[0], *a.shape[1:]), a.dtype)
                 for a in out_avals]
        outs = sharded(*dev_inputs, *zeros)
        return [
            {name: np.asarray(outs[i]).reshape(NCORES, *out_avals[i].shape)[c]
             for i, name in enumerate(out_names)}
            for c in range(NCORES)
        ]

    _CACHE["run"] = run
    _CACHE["meta"] = (in_names, out_names, out_avals, sharded, mesh)
    return run


def kernel(**inputs) -> np.ndarray:
    run = get_runner()
    in_maps = prep_inputs(inputs)
    results = run(in_maps)
    return assemble_output(results).reshape(B, S, H).astype(np.float32)
